# revision 11
# baseline (speedup 1.0000x reference)
"""Trainium2 Bass kernel for a per-joint grouped GEMM (GNN message passing).

Computes, for each batch b and joint j:
    out[b, j, :] = x[b, j, :] @ W[j] + bias[j] + joint_feats[b, j, :]
where x[b, j, :] = link_feats[b, child_idx[j]].reshape(1024).

Sharding: data-parallel over batch across 8 NeuronCores (512 rows each);
the 32 per-joint weight matrices are replicated and kept resident in SBUF.

Matmul mapping (per 128-row batch tile, per joint):
  - x is loaded as SBUF tile [k_hi=128 partitions, (b=128, k_lo=8) free] so
    every DMA element is a 32-byte contiguous DRAM run (k is contiguous in
    DRAM; 1024 k-values / 128 partitions = 8 consecutive per partition).
  - 8 accumulating fp32 matmuls (one per k_lo): lhsT = x[:, :, k_lo]
    ([k_hi, b]), rhs = W[:, j, k_lo, :] ([k_hi, cj]) -> psum [b, cj].
  - bias is folded in as a rank-1 matmul: ones[1, b].T @ bias_j[1, cj].
  - DVE adds joint_feats tile [b, cj] (layout matches psum) into the
    staged output tile, which is written back 2 MiB at a time.
"""

import os

import numpy as np

import concourse.bass as bass
import concourse.tile as tile
from concourse import bacc, mybir
from concourse.bass_utils import run_bass_kernel_spmd

F32 = mybir.dt.float32

B, NL, J, CL, S = 4096, 33, 32, 64, 16
K = CL * S          # 1024 contraction per joint
CJ = 128            # output channels per joint
NCORES = 8
BL = B // NCORES    # 512 batch rows per core
KLO = 8             # contiguous k-values per partition (32B runs)
KHI = K // KLO      # 128 partitions
BT = 128            # batch tile (psum partition dim)
NBT = BL // BT      # 4 batch tiles per core

LAST_EXEC_NS = None

_CACHE = {}


def _build_nc(child):
    nc = bacc.Bacc("TRN2", target_bir_lowering=False, debug=False)
    lf = nc.declare_dram_parameter("lf", [BL, NL * K], F32, isOutput=False)
    jf = nc.declare_dram_parameter("jf", [BL, J * CJ], F32, isOutput=False)
    w = nc.declare_dram_parameter("w", [J * K, CJ], F32, isOutput=False)
    bb = nc.declare_dram_parameter("bb", [1, J * CJ], F32, isOutput=False)
    out = nc.declare_dram_parameter("out", [BL, J * CJ], F32, isOutput=True)

    with tile.TileContext(nc) as tc:
        with (
            tc.tile_pool(name="wpool", bufs=J) as wpool,
            tc.tile_pool(name="cpool", bufs=1) as cpool,
            tc.tile_pool(name="xpool", bufs=4) as xpool,
            tc.tile_pool(name="jpool", bufs=4) as jpool,
            tc.tile_pool(name="opool", bufs=2) as opool,
            tc.tile_pool(name="psum", bufs=8, space=bass.MemorySpace.PSUM) as psum,
        ):
            # Resident weights, one tile per joint (keeps each matmul's
            # dependency set to a single DMA): wtj[p, q, c] = W[j*K + p*KLO + q, c]
            # (per partition: one 4KB contiguous DRAM run)
            wts = []
            for j in range(J):
                wtj = wpool.tile([KHI, KLO, CJ], F32, tag="wtj")
                nc.sync.dma_start(
                    wtj[:],
                    w[j * K:(j + 1) * K, :].rearrange(
                        "(p q) c -> p q c", p=KHI, q=KLO
                    ),
                )
                wts.append(wtj)

            # Bias on partition 0, and a ones row for the rank-1 bias matmul.
            bt_b = cpool.tile([1, J * CJ], F32)
            nc.sync.dma_start(bt_b[:], bb[:, :])
            ones = cpool.tile([1, BT], F32)
            nc.vector.memset(ones[:], 1.0)

            for b0 in range(NBT):
                bsl = slice(b0 * BT, (b0 + 1) * BT)
                ot = opool.tile([BT, J, CJ], F32)
                for j in range(J):
                    c = child[j]
                    # x tile: [k_hi, b, k_lo]
                    xt = xpool.tile([KHI, BT, KLO], F32)
                    nc.sync.dma_start(
                        xt[:],
                        lf[bsl, c * K:(c + 1) * K].rearrange(
                            "b (p q) -> p b q", p=KHI, q=KLO
                        ),
                    )
                    # joint_feats tile: [b, cj]
                    jt = jpool.tile([BT, CJ], F32)
                    nc.sync.dma_start(jt[:], jf[bsl, j * CJ:(j + 1) * CJ])

                    pt = psum.tile([BT, CJ], F32, tag="pt")
                    # bias: ones[1,b].T @ bias_j[1,cj]
                    nc.tensor.matmul(
                        pt[:],
                        ones[:],
                        bt_b[:, j * CJ:(j + 1) * CJ],
                        start=True,
                        stop=False,
                    )
                    for q in range(KLO):
                        nc.tensor.matmul(
                            pt[:],
                            xt[:, :, q],
                            wts[j][:, q, :],
                            start=False,
                            stop=(q == KLO - 1),
                        )
                    # out tile slice = psum + joint_feats
                    nc.vector.tensor_add(ot[:, j, :], pt[:], jt[:])
                nc.sync.dma_start(out[bsl, :], ot[:])

    nc.compile()
    return nc


def kernel(link_feats, joint_feats, W, b, child_idx):
    global LAST_EXEC_NS
    lf = np.ascontiguousarray(np.asarray(link_feats), dtype=np.float32)
    jf = np.ascontiguousarray(np.asarray(joint_feats), dtype=np.float32)
    w = np.ascontiguousarray(np.asarray(W), dtype=np.float32)
    bb = np.ascontiguousarray(np.asarray(b), dtype=np.float32)
    child = tuple(int(v) for v in np.asarray(child_idx).reshape(-1))
    assert len(child) == J

    if child not in _CACHE:
        _CACHE[child] = _build_nc(child)
    nc = _CACHE[child]

    lf2 = lf.reshape(B, NL * K)
    jf2 = jf.reshape(B, J * CJ)
    w2 = w.reshape(J * K, CJ)
    bb2 = bb.reshape(1, J * CJ)

    in_maps = []
    for core in range(NCORES):
        sl = slice(core * BL, (core + 1) * BL)
        in_maps.append(
            {
                "lf": lf2[sl],
                "jf": jf2[sl],
                "w": w2,
                "bb": bb2,
            }
        )

    trace = os.environ.get("KERNEL_TRACE", "0") == "1"
    tmpdir = os.environ.get("KERNEL_TMPDIR") or None
    if tmpdir:
        os.makedirs(tmpdir, exist_ok=True)
    res = run_bass_kernel_spmd(
        nc, in_maps, list(range(NCORES)), trace=trace, tmpdir=tmpdir
    )
    LAST_EXEC_NS = res.exec_time_ns

    out = np.concatenate([r["out"] for r in res.results], axis=0)
    return out.reshape(B, J, CJ)


# revision 12
# speedup vs baseline: 3.8074x; 3.8074x over previous
"""Trainium2 Bass kernel for a per-joint grouped GEMM (GNN message passing).

Computes, for each batch b and joint j:
    out[b, j, :] = x[b, j, :] @ W[j] + bias[j] + joint_feats[b, j, :]
where x[b, j, :] = link_feats[b, child_idx[j]].reshape(1024).

Sharding strategy: data-parallel over batch across 8 NeuronCores (512 rows
each); the 32 per-joint weight matrices are replicated and kept resident in
SBUF. As part of sharding, each core's activations are laid out k-major
(xT[j, k, b_local]) so every DMA descriptor is a >=2KB contiguous run per
SBUF partition — the TensorEngine contracts along the partition dimension,
so both matmul operands need k on partitions, and a b-major layout would
force one descriptor per 32B run (measured: 2.1M DMA packets, 1.5ms).

Device kernel, per joint j:
  - 8 accumulating fp32 matmuls: lhsT = W_j[k-chunk, cj] (stationary,
    natural layout), rhs = xT_j[k-chunk, b=512] (moving, N=512) into
    psum[cj=128, b=512] (exactly one PSUM bank).
  - bias is pre-folded into the host-prepared jfT (= joint_feats.T + b).
  - DVE adds jfT_j [cj, b] to psum -> staged SBUF tile -> one 256KB
    output DMA per joint, outT[j, cj, b_local].
Host unshards by transposing outT back to [b, j, cj].
"""

import os

import numpy as np

import concourse.bass as bass
import concourse.tile as tile
from concourse import bacc, mybir
from concourse.bass_utils import run_bass_kernel_spmd

F32 = mybir.dt.float32

B, NL, J, CL, S = 4096, 33, 32, 64, 16
K = CL * S          # 1024 contraction per joint
CJ = 128            # output channels per joint
NCORES = 8
BL = B // NCORES    # 512 batch rows per core
KC = 128            # contraction chunk (partition dim)
NKC = K // KC       # 8 chunks

LAST_EXEC_NS = None

_CACHE = {}


def _build_nc():
    nc = bacc.Bacc("TRN2", target_bir_lowering=False, debug=False)
    xt = nc.declare_dram_parameter("xt", [J * K, BL], F32, isOutput=False)
    jft = nc.declare_dram_parameter("jft", [J * CJ, BL], F32, isOutput=False)
    w = nc.declare_dram_parameter("w", [J * K, CJ], F32, isOutput=False)
    out = nc.declare_dram_parameter("out", [J * CJ, BL], F32, isOutput=True)

    with tile.TileContext(nc) as tc:
        with (
            tc.tile_pool(name="wpool", bufs=J) as wpool,
            tc.tile_pool(name="xpool", bufs=3 * NKC) as xpool,
            tc.tile_pool(name="jpool", bufs=3) as jpool,
            tc.tile_pool(name="opool", bufs=3) as opool,
            tc.tile_pool(name="psum", bufs=4, space=bass.MemorySpace.PSUM) as psum,
        ):
            # Resident weights, one tile per joint: wtj[p, q, c]
            # = W[j*K + p*KLO + q, c] -> actually [KC, NKC, CJ] with
            # wtj[p, q, c] = W[j*K + q*KC + p, c] (chunk q, row p).
            wts = []
            for j in range(J):
                wtj = wpool.tile([KC, NKC, CJ], F32, tag="wtj")
                nc.sync.dma_start(
                    wtj[:],
                    w[j * K:(j + 1) * K, :].rearrange(
                        "(q p) c -> p q c", q=NKC, p=KC
                    ),
                )
                wts.append(wtj)

            for j in range(J):
                # x chunks: [k-chunk 128, b 512], contiguous 2KB per partition
                xcs = []
                for q in range(NKC):
                    xc = xpool.tile([KC, BL], F32, tag="xc")
                    nc.sync.dma_start(
                        xc[:], xt[j * K + q * KC:j * K + (q + 1) * KC, :]
                    )
                    xcs.append(xc)
                jt = jpool.tile([CJ, BL], F32)
                nc.sync.dma_start(jt[:], jft[j * CJ:(j + 1) * CJ, :])

                pt = psum.tile([CJ, BL], F32)
                for q in range(NKC):
                    nc.tensor.matmul(
                        pt[:],
                        wts[j][:, q, :],
                        xcs[q][:],
                        start=(q == 0),
                        stop=(q == NKC - 1),
                    )
                ot = opool.tile([CJ, BL], F32)
                nc.vector.tensor_add(ot[:], pt[:], jt[:])
                nc.sync.dma_start(out[j * CJ:(j + 1) * CJ, :], ot[:])

    nc.compile()
    return nc


def kernel(link_feats, joint_feats, W, b, child_idx):
    global LAST_EXEC_NS
    lf = np.asarray(link_feats, dtype=np.float32)
    jf = np.asarray(joint_feats, dtype=np.float32)
    w = np.ascontiguousarray(np.asarray(W, dtype=np.float32).reshape(J * K, CJ))
    bb = np.asarray(b, dtype=np.float32)
    child = np.asarray(child_idx).reshape(-1).astype(np.int64)
    assert child.shape[0] == J

    if "nc" not in _CACHE:
        _CACHE["nc"] = _build_nc()
    nc = _CACHE["nc"]

    # Host-side sharding + layout: gather child links, k-major transpose,
    # fold bias into the joint-features residual.
    in_maps = []
    for core in range(NCORES):
        sl = slice(core * BL, (core + 1) * BL)
        # [BL, J, K] -> [J, K, BL]
        xc = lf[sl][:, child].reshape(BL, J, K).transpose(1, 2, 0)
        xtc = np.ascontiguousarray(xc).reshape(J * K, BL)
        # [BL, J, CJ] -> [J, CJ, BL] + bias
        jc = jf[sl].transpose(1, 2, 0) + bb[:, :, None]
        jftc = np.ascontiguousarray(jc).reshape(J * CJ, BL)
        in_maps.append({"xt": xtc, "jft": jftc, "w": w})

    trace = os.environ.get("KERNEL_TRACE", "0") == "1"
    tmpdir = os.environ.get("KERNEL_TMPDIR") or None
    if tmpdir:
        os.makedirs(tmpdir, exist_ok=True)
    res = run_bass_kernel_spmd(
        nc, in_maps, list(range(NCORES)), trace=trace, tmpdir=tmpdir
    )
    LAST_EXEC_NS = res.exec_time_ns

    # outT [J*CJ, BL] per core -> [BL, J, CJ]; concat over cores.
    parts = [
        r["out"].reshape(J, CJ, BL).transpose(2, 0, 1) for r in res.results
    ]
    return np.ascontiguousarray(np.concatenate(parts, axis=0))


# revision 13
# speedup vs baseline: 5.0668x; 1.3308x over previous
"""Trainium2 Bass kernel for a per-joint grouped GEMM (GNN message passing).

Computes, for each batch b and joint j:
    out[b, j, :] = x[b, j, :] @ W[j] + bias[j] + joint_feats[b, j, :]
where x[b, j, :] = link_feats[b, child_idx[j]].reshape(1024).

Sharding strategy: data-parallel over batch across 8 NeuronCores (512 rows
each), W replicated. As part of sharding, each core's operands are laid out
k-major and DMA-friendly: the TensorEngine contracts along the SBUF
partition dimension, so both matmul operands need k on partitions; a
b-major activation layout would force one DMA descriptor per 32B run
(measured: 2.1M packets, 1.5ms). Layouts are chosen so every DMA moves
4-16KB of contiguous DRAM per partition:
  xt  [J*KC, NKC*BL]   xt[j*KC+p, q*BL+b]  = x[b, j, q*KC+p]
  w   [J*KC, NKC*CJ]   w[j*KC+p, q*CJ+c]   = W[j, q*KC+p, c]
  jft [CJ, J*BL]       jft[c, j*BL+b]      = joint_feats[b, j, c] + bias[j, c]
  out [CJ, J*BL]       out[c, j*BL+b]      = result[b, j, c]

Device kernel, per joint j: one 2MiB x DMA + one 512KB W DMA; 8
accumulating fp32 matmuls lhsT=W-chunk [k,cj] (stationary), rhs=x-chunk
[k, b=512] (moving, N=512) into psum[cj, b] (one PSUM bank); DVE adds the
bias-folded joint_feats slice into a staged output tile; outputs written
16KB/partition per 8-joint group. W streams (read once) instead of
sitting resident, freeing SBUF for prefetch depth.
"""

import os

import numpy as np

import concourse.bass as bass
import concourse.tile as tile
from concourse import bacc, mybir
from concourse.bass_utils import run_bass_kernel_spmd

F32 = mybir.dt.float32

B, NL, J, CL, S = 4096, 33, 32, 64, 16
K = CL * S          # 1024 contraction per joint
CJ = 128            # output channels per joint
NCORES = 8
BL = B // NCORES    # 512 batch rows per core
KC = 128            # contraction chunk (partition dim)
NKC = K // KC       # 8 chunks
JG = 8              # joints per output/jf group DMA
NJG = J // JG

LAST_EXEC_NS = None

_CACHE = {}


def _build_nc():
    nc = bacc.Bacc("TRN2", target_bir_lowering=False, debug=False)
    xt = nc.declare_dram_parameter("xt", [J * KC, NKC * BL], F32, isOutput=False)
    w = nc.declare_dram_parameter("w", [J * KC, NKC * CJ], F32, isOutput=False)
    jft = nc.declare_dram_parameter("jft", [CJ, J * BL], F32, isOutput=False)
    out = nc.declare_dram_parameter("out", [CJ, J * BL], F32, isOutput=True)

    with tile.TileContext(nc) as tc:
        with (
            tc.tile_pool(name="xpool", bufs=3) as xpool,
            tc.tile_pool(name="wpool", bufs=3) as wpool,
            tc.tile_pool(name="jpool", bufs=2) as jpool,
            tc.tile_pool(name="opool", bufs=2) as opool,
            tc.tile_pool(name="psum", bufs=4, space=bass.MemorySpace.PSUM) as psum,
        ):
            for g in range(NJG):
                jt = jpool.tile([CJ, JG, BL], F32)
                nc.sync.dma_start(
                    jt[:],
                    jft[:, g * JG * BL:(g + 1) * JG * BL].rearrange(
                        "c (jj b) -> c jj b", jj=JG, b=BL
                    ),
                )
                ot = opool.tile([CJ, JG, BL], F32)
                for jj in range(JG):
                    j = g * JG + jj
                    xtile = xpool.tile([KC, NKC * BL], F32)
                    nc.sync.dma_start(xtile[:], xt[j * KC:(j + 1) * KC, :])
                    wtile = wpool.tile([KC, NKC * CJ], F32)
                    nc.sync.dma_start(wtile[:], w[j * KC:(j + 1) * KC, :])

                    pt = psum.tile([CJ, BL], F32)
                    for q in range(NKC):
                        nc.tensor.matmul(
                            pt[:],
                            wtile[:, q * CJ:(q + 1) * CJ],
                            xtile[:, q * BL:(q + 1) * BL],
                            start=(q == 0),
                            stop=(q == NKC - 1),
                        )
                    nc.vector.tensor_add(ot[:, jj, :], pt[:], jt[:, jj, :])
                nc.sync.dma_start(
                    out[:, g * JG * BL:(g + 1) * JG * BL].rearrange(
                        "c (jj b) -> c jj b", jj=JG, b=BL
                    ),
                    ot[:],
                )

    nc.compile()
    return nc


def kernel(link_feats, joint_feats, W, b, child_idx):
    global LAST_EXEC_NS
    lf = np.asarray(link_feats, dtype=np.float32)
    jf = np.asarray(joint_feats, dtype=np.float32)
    wf = np.asarray(W, dtype=np.float32)
    bb = np.asarray(b, dtype=np.float32)
    child = np.asarray(child_idx).reshape(-1).astype(np.int64)
    assert child.shape[0] == J

    if "nc" not in _CACHE:
        _CACHE["nc"] = _build_nc()
    nc = _CACHE["nc"]

    # W host layout: [J, NKC, KC, CJ] -> [J, KC, NKC, CJ] -> [J*KC, NKC*CJ]
    w2 = np.ascontiguousarray(
        wf.reshape(J, NKC, KC, CJ).transpose(0, 2, 1, 3)
    ).reshape(J * KC, NKC * CJ)

    in_maps = []
    for core in range(NCORES):
        sl = slice(core * BL, (core + 1) * BL)
        # x: [BL, J, NKC, KC] -> [J, KC, NKC, BL]
        xc = lf[sl][:, child].reshape(BL, J, NKC, KC).transpose(1, 3, 2, 0)
        xtc = np.ascontiguousarray(xc).reshape(J * KC, NKC * BL)
        # jf: [BL, J, CJ] -> [CJ, J, BL] + bias[j, c] broadcast
        jc = jf[sl].transpose(2, 1, 0) + bb.T[:, :, None]
        jftc = np.ascontiguousarray(jc).reshape(CJ, J * BL)
        in_maps.append({"xt": xtc, "jft": jftc, "w": w2})

    trace = os.environ.get("KERNEL_TRACE", "0") == "1"
    tmpdir = os.environ.get("KERNEL_TMPDIR") or None
    if tmpdir:
        os.makedirs(tmpdir, exist_ok=True)
    res = run_bass_kernel_spmd(
        nc, in_maps, list(range(NCORES)), trace=trace, tmpdir=tmpdir
    )
    LAST_EXEC_NS = res.exec_time_ns

    # out [CJ, J*BL] per core -> [BL, J, CJ]; concat over cores.
    parts = [
        r["out"].reshape(CJ, J, BL).transpose(2, 1, 0) for r in res.results
    ]
    return np.ascontiguousarray(np.concatenate(parts, axis=0))
